# revision 10
# baseline (speedup 1.0000x reference)
"""Trainium2 Bass kernel for GQA sparse (sliding-window) attention.

Problem: B=1, S=T=2048, D=4096, N=32 query heads, K=8 KV heads, H=128.
  q = x @ q_w ; k,v = x @ kv_w ; rope(q,k) ; logits = q k^T * scale
  soft-cap tanh(l/50)*50 ; causal & sliding-window(1024) mask ; softmax
  out = (probs @ v) @ out_w  summed over heads.

Sharding: one KV head + its 4 query heads per NeuronCore (8 cores).
Each core computes a partial output [S, D] (sum over its 4 heads);
the host sums the 8 partials.

v2 design (single software-pipelined phase):
  - bf16 matmul operands everywhere error-tolerant (projections, logits,
    PV, out-proj); rope arithmetic and softmax internals stay f32.
  - per chunk: two projection passes of 3 weights each (x streamed twice,
    3 PSUM accumulator banks), rope chains after both passes.
  - attention for chunk ci interleaved with out-projection of chunk ci-1
    (out-proj dd-groups emitted between attention heads) so the PE never
    waits on the ACT-heavy softmax chain.
  - softmax denominator: in-place f32 add-tree of the exp tiles on the
    (otherwise idle) GPSIMD engine + one all-ones f32r matmul per
    (head, chunk); zero-padding of exp tiles also on GPSIMD.
  - masks: 128-wide additive tiles (causal-diag / window-edge partial
    regions are exactly 128 aligned columns), applied on DVE.
  - logits matmuls over exact active windows [c0, c1).
  - output partials in bf16, host sums in f32.
"""

import numpy as np
import ml_dtypes

import concourse.bacc as bacc
import concourse.mybir as mybir
import concourse.tile as tile
from concourse.bass_utils import run_bass_kernel_spmd

# Problem constants (hardcoded per spec nn_Attention_30812095381719)
S = 2048          # sequence length (T == S)
D = 4096          # model dim
NQ = 32           # query heads
NKV = 8           # kv heads
G = NQ // NKV     # query heads per kv head = 4
H = 128           # head dim
NCORES = 8
TC = 512          # t-chunk (matmul moving free dim)
ST = 128          # s-tile (partition dim)
NCHUNK = S // TC  # 4
NST = S // ST     # 16
NDT = D // 128    # 32 contraction tiles

QUERY_SCALE = 0.08838834764831845
SOFT_CAP = 50.0
SLIDING_WINDOW = 1024
ROPE_BASE = 10000.0

F32R = mybir.dt.float32r
F32 = mybir.dt.float32
BF16 = mybir.dt.bfloat16

TANH_SCALE = float(QUERY_SCALE / SOFT_CAP)
MASK_ADD = -2000.0  # tanh in [-1,1]; exp(50*(tanh-2000)) == 0 exactly

BF = ml_dtypes.bfloat16


def _build_program(active, nmask):
    """Build the SPMD Bass program.

    active: list over t-chunk ci of list of (j, mask_idx_or_None, c0, c1, q0)
            for mask-active 128-row s-tiles; [c0, c1) = active t window,
            q0 = start of the 128-wide partial-mask region (None if full).
    nmask:  number of distinct 128-wide additive mask tiles.
    """
    nc = bacc.Bacc("TRN2", target_bir_lowering=False, debug=False)

    xT = nc.dram_tensor("xT", [D, S], BF16, kind="ExternalInput").ap()
    w_all = nc.dram_tensor("w_all", [6, 128, NDT * 128], BF16,
                           kind="ExternalInput").ap()
    wo = nc.dram_tensor("wo", [G, H, D], BF16, kind="ExternalInput").ap()
    cs = nc.dram_tensor("cs", [128, 2, NCHUNK, TC], BF16, kind="ExternalInput").ap()
    consts = nc.dram_tensor("consts", [128, 384], F32R, kind="ExternalInput").ap()
    constsb = nc.dram_tensor("constsb", [128, 128], BF16, kind="ExternalInput").ap()
    masks = nc.dram_tensor("masks", [128, max(nmask, 1), 128], BF16,
                           kind="ExternalInput").ap()
    outp = nc.dram_tensor("outp", [S, D], BF16, kind="ExternalOutput").ap()

    Tanh = mybir.ActivationFunctionType.Tanh
    Exp = mybir.ActivationFunctionType.Exp

    from contextlib import ExitStack
    with tile.TileContext(nc) as tc:
        with ExitStack() as stack:
            def pool(name, bufs=1, space="SBUF"):
                return stack.enter_context(
                    tc.tile_pool(name=name, bufs=bufs, space=space))
            constp = pool("constp")
            ropedp = pool("ropedp")
            vsbp = pool("vsbp")
            wp = pool("wp")
            csp = pool("csp")
            mp = pool("mp")
            encp = pool("encp")
            xtp = pool("xtp", bufs=4)
            evp = pool("evp", bufs=4)
            vTp = pool("vTp", bufs=2)
            rtp = pool("rtp", bufs=3)
            tpp = pool("tpp", bufs=3)
            pp = pool("pp", bufs=26)
            sump = pool("sump")
            rcp = pool("rcp", bufs=2)
            wop = pool("wop", bufs=3)
            osbp = pool("osbp", bufs=2)
            psproj = pool("psproj", space="PSUM")
            pslp = pool("psl", space="PSUM")
            psep = pool("pse", space="PSUM")
            psop = pool("pso", space="PSUM")

            ct = constp.tile([128, 384], F32R)
            ctb = constp.tile([128, 128], BF16, name="ctb", tag="ctb")
            allones = ct[:, 0:128]
            ident = ct[:, 256:384]
            swapmat = ctb

            qkr = [ropedp.tile([128, S], BF16, name=f"qkr{w}", tag=f"qkr{w}")
                   for w in range(5)]  # q0..q3, k
            v_sb = vsbp.tile([128, NST, 128], BF16)  # [s_lo, s_tile, h]
            encn = [encp.tile([128, S], BF16, name=f"encn{h}", tag=f"encn{h}")
                    for h in range(G)]

            # ---- resident small tensors -------------------------------
            cst = csp.tile([128, 2, NCHUNK, TC], BF16)
            mt = mp.tile([128, max(nmask, 1), 128], BF16)
            wts = [wp.tile([128, NDT, 128], BF16, name=f"wt{w}", tag=f"wt{w}")
                   for w in range(6)]
            w_src = [w_all[w].rearrange("p (dt h) -> p dt h", h=128)
                     for w in range(6)]

            # weight staging: dt tile 0 of pass-A weights first, then the rest
            passA = (4, 5, 0)   # k, v, q0
            passB = (1, 2, 3)
            for w in passA:
                nc.gpsimd.dma_start(out=wts[w][:, 0:1, :], in_=w_src[w][:, 0:1, :])
            for w in passA:
                nc.gpsimd.dma_start(out=wts[w][:, 1:4, :], in_=w_src[w][:, 1:4, :])
            nc.gpsimd.dma_start(out=ct, in_=consts)
            nc.gpsimd.dma_start(out=ctb, in_=constsb)
            for w in passB:
                nc.gpsimd.dma_start(out=wts[w][:, 0:4, :], in_=w_src[w][:, 0:4, :])
            nc.gpsimd.dma_start(out=cst, in_=cs)
            bounds = [4, 8, 16, 24, 32]
            for part in range(len(bounds) - 1):
                dsl_ = slice(bounds[part], bounds[part + 1])
                for w in passA + passB:
                    nc.gpsimd.dma_start(out=wts[w][:, dsl_, :],
                                        in_=w_src[w][:, dsl_, :])
                if part == 0:
                    nc.gpsimd.dma_start(out=mt, in_=masks)

            # ---------------- emission helpers ------------------------
            def emit_proj_pass(ci, ws):
                tsl = slice(ci * TC, (ci + 1) * TC)
                pss = {}
                for slot, w in enumerate(ws):
                    pss[w] = psproj.tile([128, TC], F32, name=f"pj{w}",
                                         tag=f"pj{slot}")
                for dt4 in range(NDT // 4):
                    xt = xtp.tile([128, 4, TC], BF16, name="xt", tag="xt")
                    src4 = xT[dt4 * 512:(dt4 + 1) * 512, tsl].rearrange(
                        "(k p) s -> p k s", k=4)
                    if ci == 0 and ws is passA and dt4 < 2:
                        for k in range(4):
                            nc.sync.dma_start(out=xt[:, k, :],
                                              in_=src4[:, k, :])
                    else:
                        nc.sync.dma_start(out=xt, in_=src4)
                    for k in range(4):
                        dt_i = dt4 * 4 + k
                        for w in ws:
                            nc.tensor.matmul(pss[w], wts[w][:, dt_i, :],
                                             xt[:, k, :],
                                             start=(dt_i == 0),
                                             stop=(dt_i == NDT - 1))
                return pss

            def emit_rope(ci, pss, ws):
                """Evict + rope (q/k) or transpose (v) for the given weights."""
                tsl = slice(ci * TC, (ci + 1) * TC)
                cos_t = cst[:, 0, ci, :]
                sin_t = cst[:, 1, ci, :]
                for w in ws:
                    if w == 5:
                        vT = vTp.tile([128, TC], F32R, name="vT", tag="vT")
                        nc.scalar.copy(vT, pss[w])
                        for sti in range(4):
                            st = 4 * ci + sti
                            tp = pslp.tile([128, 128], F32R, name="tp",
                                           tag=f"psl{sti % 2}")
                            nc.tensor.transpose(tp, vT[:, sti * 128:(sti + 1) * 128],
                                                ident)
                            nc.vector.tensor_copy(v_sb[:, st, :], tp)
                    else:
                        ev = evp.tile([128, TC], BF16, name="ev", tag="ev")
                        nc.scalar.copy(ev, pss[w])
                        swp = pslp.tile([128, TC], F32, name="swp",
                                        tag=f"psl{w % 2}")
                        nc.tensor.matmul(swp, swapmat, ev, start=True, stop=True)
                        m1 = rtp.tile([128, TC], BF16, name="m1", tag="m1")
                        nc.vector.tensor_mul(m1, ev, cos_t)
                        m2 = rtp.tile([128, TC], BF16, name="m2", tag="m2")
                        nc.vector.tensor_mul(m2, swp, sin_t)
                        nc.vector.tensor_add(qkr[w][:, tsl], m1, m2)

            def emit_head_logits(ci, h):
                """Logits + softmax-numerator chain for one head; returns
                (ptiles dict, denom-tree root tile)."""
                ptiles = {}
                for (j, mi, c0, c1, q0) in active[ci]:
                    csl = slice(c0, c1)
                    ps = pslp.tile([128, TC], F32, name="psl_t", tag=f"psl{j % 2}")
                    nc.tensor.matmul(
                        ps[:, csl], qkr[4][:, j * 128:(j + 1) * 128],
                        qkr[h][:, ci * TC + c0:ci * TC + c1],
                        start=True, stop=True)
                    t1 = tpp.tile([128, TC], F32, name="t1", tag="t1")
                    nc.scalar.activation(t1[:, csl], ps[:, csl], Tanh,
                                         scale=TANH_SCALE)
                    if mi is not None:
                        nc.vector.tensor_add(t1[:, q0:q0 + 128],
                                             t1[:, q0:q0 + 128], mt[:, mi, :])
                    pt = pp.tile([128, TC], BF16, name="pt", tag="pt")
                    if c0 > 0:
                        nc.gpsimd.memset(pt[:, 0:c0], 0.0)
                    if c1 < TC:
                        nc.gpsimd.memset(pt[:, c1:TC], 0.0)
                    nc.scalar.activation(pt[:, csl], t1[:, csl], Exp,
                                         scale=SOFT_CAP)
                    ptiles[j] = pt
                # denominator: serial in-place accumulation on DVE, paced
                # by ACT exp production (each add consumes p_j as it lands)
                leaves = [ptiles[j] for j, _, _, _, _ in active[ci]]
                run = sump.tile([128, TC], F32R, name=f"run{h}",
                                tag=f"run{h % 2}")
                nc.vector.tensor_add(run, leaves[0], leaves[1])
                for p in leaves[2:]:
                    nc.vector.tensor_add(run, run, p)
                return ptiles, run

            def emit_head_finalize(ci, h, ptiles, run):
                tsl = slice(ci * TC, (ci + 1) * TC)
                js = [j for j, _, _, _, _ in active[ci]]
                dps = pslp.tile([128, TC], F32, name="dps", tag=f"psl{h % 2}")
                nc.tensor.matmul(dps, allones, run, start=True, stop=True)
                rec = rcp.tile([128, TC], F32, name="rec", tag="rec")
                nc.vector.reciprocal_approx_fast(out=rec, in_=dps)
                eps = psep.tile([128, TC], F32, name="eps", tag="eps")
                for idx, j in enumerate(js):
                    nc.tensor.matmul(eps, v_sb[:, j, :], ptiles[j],
                                     start=(idx == 0), stop=(idx == len(js) - 1))
                nc.vector.tensor_mul(encn[h][:, tsl], eps, rec)

            def emit_outproj_group(cp, dd):
                """Out-projection of chunk cp for output-dim slice dd."""
                dsl = slice(dd * TC, (dd + 1) * TC)
                w4 = wop.tile([128, G, TC], BF16, name="w_t", tag="wo")
                nc.sync.dma_start(
                    out=w4, in_=wo[:, :, dsl].rearrange("g p d -> p g d"))
                ot4 = osbp.tile([128, 4, TC], BF16, name="ot", tag="ot")
                for ti, tt in enumerate(range(4 * cp, 4 * cp + 4)):
                    ps = psop.tile([128, TC], F32, name="pso_t", tag=f"pso{tt % 2}")
                    for h in range(G):
                        nc.tensor.matmul(
                            ps, encn[h][:, tt * 128:(tt + 1) * 128],
                            w4[:, h, :], start=(h == 0), stop=(h == G - 1))
                    if ti % 2 == 0:
                        nc.vector.tensor_copy(ot4[:, ti, :], ps)
                    else:
                        nc.scalar.copy(ot4[:, ti, :], ps)
                    if ti == 1 or ti == 3:
                        half = slice(ti - 1, ti + 1)
                        dst = outp[(4 * cp + ti - 1) * 128:(4 * cp + ti + 1) * 128,
                                   dsl].rearrange("(tt p) d -> p tt d", tt=2)
                        nc.sync.dma_start(out=dst, in_=ot4[:, half, :])

            def emit_proj(ci):
                pssA = emit_proj_pass(ci, passA)
                pssB = emit_proj_pass(ci, passB)
                emit_rope(ci, pssA, passA)
                emit_rope(ci, pssB, passB)

            def emit_attn(ci, filler):
                """Attention for chunk ci; filler = list of thunks emitted
                between heads to keep the PE busy during softmax chains."""
                fi = 0
                prev = None
                for h in range(G):
                    ptiles, run = emit_head_logits(ci, h)
                    for _ in range(2):
                        if fi < len(filler):
                            filler[fi]()
                            fi += 1
                    if prev is not None:
                        emit_head_finalize(ci, h - 1, *prev)
                    prev = (ptiles, run)
                emit_head_finalize(ci, G - 1, *prev)
                while fi < len(filler):
                    filler[fi]()
                    fi += 1

            # ---------------- program schedule -------------------------
            emit_proj(0)
            emit_proj(1)
            emit_attn(0, [])
            emit_proj(2)
            emit_attn(1, [lambda dd=dd: emit_outproj_group(0, dd)
                          for dd in range(8)])
            emit_proj(3)
            emit_attn(2, [lambda dd=dd: emit_outproj_group(1, dd)
                          for dd in range(8)])
            emit_attn(3, [lambda dd=dd: emit_outproj_group(2, dd)
                          for dd in range(8)])
            for dd in range(8):
                emit_outproj_group(3, dd)

    nc.compile()
    return nc


def _host_prep(x, segment_pos, attn_mask):
    """Host-side preprocessing shared by all cores."""
    xT = np.ascontiguousarray(x[0].T).astype(BF)

    # rope tables, emulating the reference's float32 computation
    pos = segment_pos[0].astype(np.float32)                      # [S]
    fraction = (2.0 * np.arange(H // 2, dtype=np.float32)
                / np.float32(H)).astype(np.float32)
    timescale = (np.float32(ROPE_BASE) ** fraction).astype(np.float32)
    sinusoid = (pos[None, :] / timescale[:, None]).astype(np.float32)  # [64, S]
    cosT = np.cos(sinusoid).astype(np.float32)
    sinT = np.sin(sinusoid).astype(np.float32)
    cos2 = np.concatenate([cosT, cosT], axis=0)                  # [128, S]
    sin2 = np.concatenate([-sinT, sinT], axis=0)                 # [128, S]
    cs = np.ascontiguousarray(
        np.stack([cos2.reshape(128, NCHUNK, TC),
                  sin2.reshape(128, NCHUNK, TC)], axis=1)).astype(BF)

    # combined mask [T, S]
    cache_positions = np.arange(S, dtype=np.int64)[None, :]
    sp = segment_pos[0].astype(np.int64)[:, None]
    sliding = (cache_positions > sp - SLIDING_WINDOW) & \
              (cache_positions < sp + SLIDING_WINDOW)
    combined = np.asarray(attn_mask[0], dtype=bool) & sliding    # [T, S]

    # block classification at (128 s) x (512 t) granularity
    active = []
    mask_list = []
    mask_index = {}
    for ci in range(NCHUNK):
        row = []
        for j in range(NST):
            sub = combined[ci * TC:(ci + 1) * TC, j * ST:(j + 1) * ST]  # [t, s]
            if not sub.any():
                continue
            colact = sub.any(axis=1)              # [t]
            c0 = int(np.argmax(colact)) & ~7
            c1 = min(TC, (int(TC - np.argmax(colact[::-1])) + 7) & ~7)
            if sub[c0:c1].all():
                row.append((j, None, c0, c1, None))
                continue
            # partial columns: not all-active within [c0, c1)
            partial = ~sub[c0:c1].all(axis=1)     # [c1-c0]
            pidx = np.nonzero(partial)[0]
            q0 = (c0 + int(pidx[0])) & ~7
            q1 = c0 + int(pidx[-1]) + 1
            assert q1 - q0 <= 128, f"mask region too wide: {q0}..{q1}"
            q0 = min(q0, TC - 128)
            madd = np.where(sub.T[:, q0:q0 + 128], np.float32(0.0),
                            np.float32(MASK_ADD)).astype(BF)     # [s, 128]
            key = madd.tobytes()
            if key not in mask_index:
                mask_index[key] = len(mask_list)
                mask_list.append(madd)
            row.append((j, mask_index[key], c0, c1, q0))
        assert row, f"t-chunk {ci} attends to nothing"
        assert len(row) >= 2
        active.append(row)
    nmask = len(mask_list)
    if nmask:
        masks_host = np.ascontiguousarray(np.stack(mask_list, axis=1))  # [128,n,128]
    else:
        masks_host = np.zeros((128, 1, 128), dtype=BF)

    # consts: allones | swapmat | identity (f32r)
    allones = np.ones((128, 128), dtype=np.float32)
    swapmat = np.zeros((128, 128), dtype=np.float32)
    idx = np.arange(128)
    swapmat[idx, (idx + 64) % 128] = 1.0
    identity = np.eye(128, dtype=np.float32)
    consts = np.ascontiguousarray(
        np.concatenate([allones, swapmat, identity], axis=1))    # [128, 384]

    swapb = swapmat.astype(BF)
    return xT, cs, active, nmask, masks_host, consts, swapb


def _core_weights(q_w, kv_w, out_w, c):
    qsel = np.asarray(q_w[G * c:G * (c + 1)], dtype=np.float32)   # [4,D,H]
    ksel = np.asarray(kv_w[0, c], dtype=np.float32)               # [D,H]
    vsel = np.asarray(kv_w[1, c], dtype=np.float32)               # [D,H]
    w6 = np.stack([qsel[0], qsel[1], qsel[2], qsel[3], ksel, vsel], axis=0)
    # [6, D, H] -> [6, 128(p), NDT*128] with (dt, h) contiguous per partition
    w_all_host = np.ascontiguousarray(
        w6.reshape(6, NDT, 128, 128).transpose(0, 2, 1, 3)
        .reshape(6, 128, NDT * 128)).astype(BF)
    wo_host = np.ascontiguousarray(
        np.asarray(out_w[G * c:G * (c + 1)], dtype=np.float32)).astype(BF)
    return w_all_host, wo_host


def kernel(x, segment_pos, attn_mask, q_w, kv_w, out_w, _trace=False, _repeat=1):
    x = np.asarray(x)
    segment_pos = np.asarray(segment_pos)
    attn_mask = np.asarray(attn_mask)
    q_w = np.asarray(q_w)
    kv_w = np.asarray(kv_w)
    out_w = np.asarray(out_w)
    assert x.shape == (1, S, D) and q_w.shape == (NQ, D, H), \
        f"kernel hardcoded for {(1, S, D)}, got {x.shape}"

    xT, cs, active, nmask, masks_host, consts, swapb = _host_prep(
        x, segment_pos, attn_mask)

    nc = _build_program(active, nmask)

    in_maps = []
    for c in range(NCORES):
        w_all_host, wo_host = _core_weights(q_w, kv_w, out_w, c)
        in_maps.append({
            "xT": xT, "w_all": w_all_host, "wo": wo_host, "cs": cs,
            "consts": consts, "constsb": swapb, "masks": masks_host,
        })

    res = run_bass_kernel_spmd(nc, in_maps, list(range(NCORES)), trace=_trace)
    kernel._last_exec_ns = res.exec_time_ns
    kernel._all_exec_ns = [res.exec_time_ns]
    for _ in range(_repeat - 1):
        r2 = run_bass_kernel_spmd(nc, in_maps, list(range(NCORES)), trace=_trace)
        kernel._all_exec_ns.append(r2.exec_time_ns)
        res = r2
    if _repeat > 1 and any(t for t in kernel._all_exec_ns if t):
        kernel._last_exec_ns = min(t for t in kernel._all_exec_ns if t)

    out = res.results[0]["outp"].astype(np.float32)
    for c in range(1, NCORES):
        out += res.results[c]["outp"].astype(np.float32)
    return out[None]  # [1, S, D]


kernel._last_exec_ns = None


# revision 11
# speedup vs baseline: 1.0102x; 1.0102x over previous
"""Trainium2 Bass kernel for GQA sparse (sliding-window) attention.

Problem: B=1, S=T=2048, D=4096, N=32 query heads, K=8 KV heads, H=128.
  q = x @ q_w ; k,v = x @ kv_w ; rope(q,k) ; logits = q k^T * scale
  soft-cap tanh(l/50)*50 ; causal & sliding-window(1024) mask ; softmax
  out = (probs @ v) @ out_w  summed over heads.

Sharding: one KV head + its 4 query heads per NeuronCore (8 cores).
Each core computes a partial output [S, D] (sum over its 4 heads);
the host sums the 8 partials.

v2 design (single software-pipelined phase):
  - bf16 matmul operands everywhere error-tolerant (projections, logits,
    PV, out-proj); rope arithmetic and softmax internals stay f32.
  - per chunk: two projection passes of 3 weights each (x streamed twice,
    3 PSUM accumulator banks), rope chains after both passes.
  - attention for chunk ci interleaved with out-projection of chunk ci-1
    (out-proj dd-groups emitted between attention heads) so the PE never
    waits on the ACT-heavy softmax chain.
  - softmax denominator: in-place f32 add-tree of the exp tiles on the
    (otherwise idle) GPSIMD engine + one all-ones f32r matmul per
    (head, chunk); zero-padding of exp tiles also on GPSIMD.
  - masks: 128-wide additive tiles (causal-diag / window-edge partial
    regions are exactly 128 aligned columns), applied on DVE.
  - logits matmuls over exact active windows [c0, c1).
  - output partials in bf16, host sums in f32.
"""

import numpy as np
import ml_dtypes

import concourse.bacc as bacc
import concourse.mybir as mybir
import concourse.tile as tile
from concourse.bass_utils import run_bass_kernel_spmd

# Problem constants (hardcoded per spec nn_Attention_30812095381719)
S = 2048          # sequence length (T == S)
D = 4096          # model dim
NQ = 32           # query heads
NKV = 8           # kv heads
G = NQ // NKV     # query heads per kv head = 4
H = 128           # head dim
NCORES = 8
TC = 512          # t-chunk (matmul moving free dim)
ST = 128          # s-tile (partition dim)
NCHUNK = S // TC  # 4
NST = S // ST     # 16
NDT = D // 128    # 32 contraction tiles

QUERY_SCALE = 0.08838834764831845
SOFT_CAP = 50.0
SLIDING_WINDOW = 1024
ROPE_BASE = 10000.0

F32R = mybir.dt.float32r
F32 = mybir.dt.float32
BF16 = mybir.dt.bfloat16

TANH_SCALE = float(QUERY_SCALE / SOFT_CAP)
MASK_ADD = -2000.0  # tanh in [-1,1]; exp(50*(tanh-2000)) == 0 exactly

BF = ml_dtypes.bfloat16


def _build_program(active, nmask):
    """Build the SPMD Bass program.

    active: list over t-chunk ci of list of (j, mask_idx_or_None, c0, c1, q0)
            for mask-active 128-row s-tiles; [c0, c1) = active t window,
            q0 = start of the 128-wide partial-mask region (None if full).
    nmask:  number of distinct 128-wide additive mask tiles.
    """
    nc = bacc.Bacc("TRN2", target_bir_lowering=False, debug=False)

    xT = nc.dram_tensor("xT", [D, S], BF16, kind="ExternalInput").ap()
    w_all = nc.dram_tensor("w_all", [6, 128, NDT * 128], BF16,
                           kind="ExternalInput").ap()
    wo = nc.dram_tensor("wo", [G, H, D], BF16, kind="ExternalInput").ap()
    cs = nc.dram_tensor("cs", [128, 2, NCHUNK, TC], BF16, kind="ExternalInput").ap()
    consts = nc.dram_tensor("consts", [128, 384], F32R, kind="ExternalInput").ap()
    constsb = nc.dram_tensor("constsb", [128, 128], BF16, kind="ExternalInput").ap()
    masks = nc.dram_tensor("masks", [128, max(nmask, 1), 128], BF16,
                           kind="ExternalInput").ap()
    outp = nc.dram_tensor("outp", [S, D], BF16, kind="ExternalOutput").ap()

    Tanh = mybir.ActivationFunctionType.Tanh
    Exp = mybir.ActivationFunctionType.Exp

    from contextlib import ExitStack
    with tile.TileContext(nc) as tc:
        with ExitStack() as stack:
            def pool(name, bufs=1, space="SBUF"):
                return stack.enter_context(
                    tc.tile_pool(name=name, bufs=bufs, space=space))
            constp = pool("constp")
            ropedp = pool("ropedp")
            vsbp = pool("vsbp")
            wp = pool("wp")
            csp = pool("csp")
            mp = pool("mp")
            encp = pool("encp")
            xtp = pool("xtp", bufs=4)
            evp = pool("evp", bufs=4)
            vTp = pool("vTp", bufs=2)
            rtp = pool("rtp", bufs=3)
            tpp = pool("tpp", bufs=3)
            pp = pool("pp", bufs=26)
            sump = pool("sump")
            rcp = pool("rcp", bufs=2)
            wop = pool("wop", bufs=3)
            osbp = pool("osbp", bufs=2)
            psproj = pool("psproj", space="PSUM")
            pslp = pool("psl", space="PSUM")
            psep = pool("pse", space="PSUM")
            psop = pool("pso", space="PSUM")

            ct = constp.tile([128, 384], F32R)
            ctb = constp.tile([128, 128], BF16, name="ctb", tag="ctb")
            allones = ct[:, 0:128]
            ident = ct[:, 256:384]
            swapmat = ctb

            qkr = [ropedp.tile([128, S], BF16, name=f"qkr{w}", tag=f"qkr{w}")
                   for w in range(5)]  # q0..q3, k
            v_sb = vsbp.tile([128, NST, 128], BF16)  # [s_lo, s_tile, h]
            encn = [encp.tile([128, S], BF16, name=f"encn{h}", tag=f"encn{h}")
                    for h in range(G)]

            # ---- resident small tensors -------------------------------
            cst = csp.tile([128, 2, NCHUNK, TC], BF16)
            mt = mp.tile([128, max(nmask, 1), 128], BF16)
            wts = [wp.tile([128, NDT, 128], BF16, name=f"wt{w}", tag=f"wt{w}")
                   for w in range(6)]
            w_src = [w_all[w].rearrange("p (dt h) -> p dt h", h=128)
                     for w in range(6)]

            # weight staging: dt tile 0 of pass-A weights first, then the rest
            passA = (4, 5, 0)   # k, v, q0
            passB = (1, 2, 3)
            for w in passA:
                nc.gpsimd.dma_start(out=wts[w][:, 0:1, :], in_=w_src[w][:, 0:1, :])
            boundsA = [1, 2, 4, 8, 16, 24, 32]
            for part in range(len(boundsA) - 1):
                dsl_ = slice(boundsA[part], boundsA[part + 1])
                for w in passA:
                    nc.gpsimd.dma_start(out=wts[w][:, dsl_, :],
                                        in_=w_src[w][:, dsl_, :])
            nc.gpsimd.dma_start(out=ctb, in_=constsb)
            nc.gpsimd.dma_start(out=cst, in_=cs)
            nc.gpsimd.dma_start(out=ct, in_=consts)
            boundsB = [0, 4, 8, 16, 24, 32]
            for part in range(len(boundsB) - 1):
                dsl_ = slice(boundsB[part], boundsB[part + 1])
                for w in passB:
                    nc.gpsimd.dma_start(out=wts[w][:, dsl_, :],
                                        in_=w_src[w][:, dsl_, :])
            nc.gpsimd.dma_start(out=mt, in_=masks)

            # ---------------- emission helpers ------------------------
            def emit_proj_pass(ci, ws):
                tsl = slice(ci * TC, (ci + 1) * TC)
                pss = {}
                for slot, w in enumerate(ws):
                    pss[w] = psproj.tile([128, TC], F32, name=f"pj{w}",
                                         tag=f"pj{slot}")
                for dt4 in range(NDT // 4):
                    xt = xtp.tile([128, 4, TC], BF16, name="xt", tag="xt")
                    src4 = xT[dt4 * 512:(dt4 + 1) * 512, tsl].rearrange(
                        "(k p) s -> p k s", k=4)
                    nc.sync.dma_start(out=xt, in_=src4)
                    for k in range(4):
                        dt_i = dt4 * 4 + k
                        for w in ws:
                            nc.tensor.matmul(pss[w], wts[w][:, dt_i, :],
                                             xt[:, k, :],
                                             start=(dt_i == 0),
                                             stop=(dt_i == NDT - 1))
                return pss

            def emit_rope(ci, pss, ws):
                """Evict + rope (q/k) or transpose (v) for the given weights."""
                tsl = slice(ci * TC, (ci + 1) * TC)
                cos_t = cst[:, 0, ci, :]
                sin_t = cst[:, 1, ci, :]
                for w in ws:
                    if w == 5:
                        vT = vTp.tile([128, TC], F32R, name="vT", tag="vT")
                        nc.scalar.copy(vT, pss[w])
                        for sti in range(4):
                            st = 4 * ci + sti
                            tp = pslp.tile([128, 128], F32R, name="tp",
                                           tag=f"psl{sti % 2}")
                            nc.tensor.transpose(tp, vT[:, sti * 128:(sti + 1) * 128],
                                                ident)
                            nc.vector.tensor_copy(v_sb[:, st, :], tp)
                    else:
                        ev = evp.tile([128, TC], BF16, name="ev", tag="ev")
                        nc.scalar.copy(ev, pss[w])
                        swp = pslp.tile([128, TC], F32, name="swp",
                                        tag=f"psl{w % 2}")
                        nc.tensor.matmul(swp, swapmat, ev, start=True, stop=True)
                        m1 = rtp.tile([128, TC], BF16, name="m1", tag="m1")
                        nc.vector.tensor_mul(m1, ev, cos_t)
                        m2 = rtp.tile([128, TC], BF16, name="m2", tag="m2")
                        nc.vector.tensor_mul(m2, swp, sin_t)
                        nc.vector.tensor_add(qkr[w][:, tsl], m1, m2)

            def emit_head_logits(ci, h):
                """Logits + softmax-numerator chain for one head; returns
                (ptiles dict, denom-tree root tile)."""
                ptiles = {}
                for (j, mi, c0, c1, q0) in active[ci]:
                    csl = slice(c0, c1)
                    ps = pslp.tile([128, TC], F32, name="psl_t", tag=f"psl{j % 2}")
                    nc.tensor.matmul(
                        ps[:, csl], qkr[4][:, j * 128:(j + 1) * 128],
                        qkr[h][:, ci * TC + c0:ci * TC + c1],
                        start=True, stop=True)
                    t1 = tpp.tile([128, TC], F32, name="t1", tag="t1")
                    nc.scalar.activation(t1[:, csl], ps[:, csl], Tanh,
                                         scale=TANH_SCALE)
                    if mi is not None:
                        nc.vector.tensor_add(t1[:, q0:q0 + 128],
                                             t1[:, q0:q0 + 128], mt[:, mi, :])
                    pt = pp.tile([128, TC], BF16, name="pt", tag="pt")
                    if c0 > 0:
                        nc.gpsimd.memset(pt[:, 0:c0], 0.0)
                    if c1 < TC:
                        nc.gpsimd.memset(pt[:, c1:TC], 0.0)
                    nc.scalar.activation(pt[:, csl], t1[:, csl], Exp,
                                         scale=SOFT_CAP)
                    ptiles[j] = pt
                # denominator: serial in-place accumulation on DVE, paced
                # by ACT exp production (each add consumes p_j as it lands)
                leaves = [ptiles[j] for j, _, _, _, _ in active[ci]]
                run = sump.tile([128, TC], F32R, name=f"run{h}",
                                tag=f"run{h % 2}")
                nc.vector.tensor_add(run, leaves[0], leaves[1])
                for p in leaves[2:]:
                    nc.vector.tensor_add(run, run, p)
                return ptiles, run

            def emit_head_finalize(ci, h, ptiles, run):
                tsl = slice(ci * TC, (ci + 1) * TC)
                js = [j for j, _, _, _, _ in active[ci]]
                dps = pslp.tile([128, TC], F32, name="dps", tag=f"psl{h % 2}")
                nc.tensor.matmul(dps, allones, run, start=True, stop=True)
                rec = rcp.tile([128, TC], F32, name="rec", tag="rec")
                nc.vector.reciprocal_approx_fast(out=rec, in_=dps)
                eps = psep.tile([128, TC], F32, name="eps", tag="eps")
                for idx, j in enumerate(js):
                    nc.tensor.matmul(eps, v_sb[:, j, :], ptiles[j],
                                     start=(idx == 0), stop=(idx == len(js) - 1))
                nc.vector.tensor_mul(encn[h][:, tsl], eps, rec)

            def prefetch_wo(dd):
                dsl = slice(dd * TC, (dd + 1) * TC)
                w4 = wop.tile([128, G, TC], BF16, name="w_t", tag="wo")
                nc.sync.dma_start(
                    out=w4, in_=wo[:, :, dsl].rearrange("g p d -> p g d"))
                return w4

            def emit_outproj_group(cp, dd, w4):
                """Out-projection of chunk cp for output-dim slice dd."""
                dsl = slice(dd * TC, (dd + 1) * TC)
                ot4 = osbp.tile([128, 4, TC], BF16, name="ot", tag="ot")
                for ti, tt in enumerate(range(4 * cp, 4 * cp + 4)):
                    ps = psop.tile([128, TC], F32, name="pso_t", tag=f"pso{tt % 2}")
                    for h in range(G):
                        nc.tensor.matmul(
                            ps, encn[h][:, tt * 128:(tt + 1) * 128],
                            w4[:, h, :], start=(h == 0), stop=(h == G - 1))
                    if ti % 2 == 0:
                        nc.vector.tensor_copy(ot4[:, ti, :], ps)
                    else:
                        nc.scalar.copy(ot4[:, ti, :], ps)
                    if ti == 1 or ti == 3:
                        half = slice(ti - 1, ti + 1)
                        dst = outp[(4 * cp + ti - 1) * 128:(4 * cp + ti + 1) * 128,
                                   dsl].rearrange("(tt p) d -> p tt d", tt=2)
                        nc.sync.dma_start(out=dst, in_=ot4[:, half, :])

            def emit_proj(ci):
                pssA = emit_proj_pass(ci, passA)
                pssB = emit_proj_pass(ci, passB)
                emit_rope(ci, pssA, passA)
                emit_rope(ci, pssB, passB)

            def emit_attn(ci, cp):
                """Attention for chunk ci, with out-projection of chunk cp
                (None for no filler) interleaved between heads to keep the
                PE busy during softmax chains."""
                w4s = {}
                if cp is not None:
                    for dd in range(2):
                        w4s[dd] = prefetch_wo(dd)
                fi = 0
                prev = None
                for h in range(G):
                    ptiles, run = emit_head_logits(ci, h)
                    for _ in range(2):
                        if cp is not None and fi < 8:
                            if fi + 2 < 8:
                                w4s[fi + 2] = prefetch_wo(fi + 2)
                            emit_outproj_group(cp, fi, w4s.pop(fi))
                            fi += 1
                    if prev is not None:
                        emit_head_finalize(ci, h - 1, *prev)
                    prev = (ptiles, run)
                emit_head_finalize(ci, G - 1, *prev)
                while cp is not None and fi < 8:
                    emit_outproj_group(cp, fi, w4s.pop(fi))
                    fi += 1

            # ---------------- program schedule -------------------------
            emit_proj(0)
            emit_proj(1)
            emit_attn(0, None)
            emit_proj(2)
            emit_attn(1, 0)
            emit_proj(3)
            emit_attn(2, 1)
            emit_attn(3, 2)
            w4a = prefetch_wo(0)
            w4b = prefetch_wo(1)
            for dd in range(8):
                nxt = prefetch_wo(dd + 2) if dd + 2 < 8 else None
                emit_outproj_group(3, dd, w4a)
                w4a, w4b = w4b, nxt

    nc.compile()
    return nc


def _host_prep(x, segment_pos, attn_mask):
    """Host-side preprocessing shared by all cores."""
    xT = np.ascontiguousarray(x[0].T).astype(BF)

    # rope tables, emulating the reference's float32 computation
    pos = segment_pos[0].astype(np.float32)                      # [S]
    fraction = (2.0 * np.arange(H // 2, dtype=np.float32)
                / np.float32(H)).astype(np.float32)
    timescale = (np.float32(ROPE_BASE) ** fraction).astype(np.float32)
    sinusoid = (pos[None, :] / timescale[:, None]).astype(np.float32)  # [64, S]
    cosT = np.cos(sinusoid).astype(np.float32)
    sinT = np.sin(sinusoid).astype(np.float32)
    cos2 = np.concatenate([cosT, cosT], axis=0)                  # [128, S]
    sin2 = np.concatenate([-sinT, sinT], axis=0)                 # [128, S]
    cs = np.ascontiguousarray(
        np.stack([cos2.reshape(128, NCHUNK, TC),
                  sin2.reshape(128, NCHUNK, TC)], axis=1)).astype(BF)

    # combined mask [T, S]
    cache_positions = np.arange(S, dtype=np.int64)[None, :]
    sp = segment_pos[0].astype(np.int64)[:, None]
    sliding = (cache_positions > sp - SLIDING_WINDOW) & \
              (cache_positions < sp + SLIDING_WINDOW)
    combined = np.asarray(attn_mask[0], dtype=bool) & sliding    # [T, S]

    # block classification at (128 s) x (512 t) granularity
    active = []
    mask_list = []
    mask_index = {}
    for ci in range(NCHUNK):
        row = []
        for j in range(NST):
            sub = combined[ci * TC:(ci + 1) * TC, j * ST:(j + 1) * ST]  # [t, s]
            if not sub.any():
                continue
            colact = sub.any(axis=1)              # [t]
            c0 = int(np.argmax(colact)) & ~7
            c1 = min(TC, (int(TC - np.argmax(colact[::-1])) + 7) & ~7)
            if sub[c0:c1].all():
                row.append((j, None, c0, c1, None))
                continue
            # partial columns: not all-active within [c0, c1)
            partial = ~sub[c0:c1].all(axis=1)     # [c1-c0]
            pidx = np.nonzero(partial)[0]
            q0 = (c0 + int(pidx[0])) & ~7
            q1 = c0 + int(pidx[-1]) + 1
            assert q1 - q0 <= 128, f"mask region too wide: {q0}..{q1}"
            q0 = min(q0, TC - 128)
            madd = np.where(sub.T[:, q0:q0 + 128], np.float32(0.0),
                            np.float32(MASK_ADD)).astype(BF)     # [s, 128]
            key = madd.tobytes()
            if key not in mask_index:
                mask_index[key] = len(mask_list)
                mask_list.append(madd)
            row.append((j, mask_index[key], c0, c1, q0))
        assert row, f"t-chunk {ci} attends to nothing"
        assert len(row) >= 2
        active.append(row)
    nmask = len(mask_list)
    if nmask:
        masks_host = np.ascontiguousarray(np.stack(mask_list, axis=1))  # [128,n,128]
    else:
        masks_host = np.zeros((128, 1, 128), dtype=BF)

    # consts: allones | swapmat | identity (f32r)
    allones = np.ones((128, 128), dtype=np.float32)
    swapmat = np.zeros((128, 128), dtype=np.float32)
    idx = np.arange(128)
    swapmat[idx, (idx + 64) % 128] = 1.0
    identity = np.eye(128, dtype=np.float32)
    consts = np.ascontiguousarray(
        np.concatenate([allones, swapmat, identity], axis=1))    # [128, 384]

    swapb = swapmat.astype(BF)
    return xT, cs, active, nmask, masks_host, consts, swapb


def _core_weights(q_w, kv_w, out_w, c):
    qsel = np.asarray(q_w[G * c:G * (c + 1)], dtype=np.float32)   # [4,D,H]
    ksel = np.asarray(kv_w[0, c], dtype=np.float32)               # [D,H]
    vsel = np.asarray(kv_w[1, c], dtype=np.float32)               # [D,H]
    w6 = np.stack([qsel[0], qsel[1], qsel[2], qsel[3], ksel, vsel], axis=0)
    # [6, D, H] -> [6, 128(p), NDT*128] with (dt, h) contiguous per partition
    w_all_host = np.ascontiguousarray(
        w6.reshape(6, NDT, 128, 128).transpose(0, 2, 1, 3)
        .reshape(6, 128, NDT * 128)).astype(BF)
    wo_host = np.ascontiguousarray(
        np.asarray(out_w[G * c:G * (c + 1)], dtype=np.float32)).astype(BF)
    return w_all_host, wo_host


def kernel(x, segment_pos, attn_mask, q_w, kv_w, out_w, _trace=False, _repeat=1):
    x = np.asarray(x)
    segment_pos = np.asarray(segment_pos)
    attn_mask = np.asarray(attn_mask)
    q_w = np.asarray(q_w)
    kv_w = np.asarray(kv_w)
    out_w = np.asarray(out_w)
    assert x.shape == (1, S, D) and q_w.shape == (NQ, D, H), \
        f"kernel hardcoded for {(1, S, D)}, got {x.shape}"

    xT, cs, active, nmask, masks_host, consts, swapb = _host_prep(
        x, segment_pos, attn_mask)

    nc = _build_program(active, nmask)

    in_maps = []
    for c in range(NCORES):
        w_all_host, wo_host = _core_weights(q_w, kv_w, out_w, c)
        in_maps.append({
            "xT": xT, "w_all": w_all_host, "wo": wo_host, "cs": cs,
            "consts": consts, "constsb": swapb, "masks": masks_host,
        })

    res = run_bass_kernel_spmd(nc, in_maps, list(range(NCORES)), trace=_trace)
    kernel._last_exec_ns = res.exec_time_ns
    kernel._all_exec_ns = [res.exec_time_ns]
    for _ in range(_repeat - 1):
        r2 = run_bass_kernel_spmd(nc, in_maps, list(range(NCORES)), trace=_trace)
        kernel._all_exec_ns.append(r2.exec_time_ns)
        res = r2
    if _repeat > 1 and any(t for t in kernel._all_exec_ns if t):
        kernel._last_exec_ns = min(t for t in kernel._all_exec_ns if t)

    out = res.results[0]["outp"].astype(np.float32)
    for c in range(1, NCORES):
        out += res.results[c]["outp"].astype(np.float32)
    return out[None]  # [1, S, D]


kernel._last_exec_ns = None


# revision 13
# speedup vs baseline: 1.0154x; 1.0052x over previous
"""Trainium2 Bass kernel for GQA sparse (sliding-window) attention.

Problem: B=1, S=T=2048, D=4096, N=32 query heads, K=8 KV heads, H=128.
  q = x @ q_w ; k,v = x @ kv_w ; rope(q,k) ; logits = q k^T * scale
  soft-cap tanh(l/50)*50 ; causal & sliding-window(1024) mask ; softmax
  out = (probs @ v) @ out_w  summed over heads.

Sharding: one KV head + its 4 query heads per NeuronCore (8 cores).
Each core computes a partial output [S, D] (sum over its 4 heads);
the host sums the 8 partials.

v2 design (single software-pipelined phase):
  - bf16 matmul operands everywhere error-tolerant (projections, logits,
    PV, out-proj); rope arithmetic and softmax internals stay f32.
  - per chunk: two projection passes of 3 weights each (x streamed twice,
    3 PSUM accumulator banks), rope chains after both passes.
  - attention for chunk ci interleaved with out-projection of chunk ci-1
    (out-proj dd-groups emitted between attention heads) so the PE never
    waits on the ACT-heavy softmax chain.
  - softmax denominator: in-place f32 add-tree of the exp tiles on the
    (otherwise idle) GPSIMD engine + one all-ones f32r matmul per
    (head, chunk); zero-padding of exp tiles also on GPSIMD.
  - masks: 128-wide additive tiles (causal-diag / window-edge partial
    regions are exactly 128 aligned columns), applied on DVE.
  - logits matmuls over exact active windows [c0, c1).
  - output partials in bf16, host sums in f32.
"""

import numpy as np
import ml_dtypes

import concourse.bacc as bacc
import concourse.mybir as mybir
import concourse.tile as tile
from concourse.bass_utils import run_bass_kernel_spmd

# Problem constants (hardcoded per spec nn_Attention_30812095381719)
S = 2048          # sequence length (T == S)
D = 4096          # model dim
NQ = 32           # query heads
NKV = 8           # kv heads
G = NQ // NKV     # query heads per kv head = 4
H = 128           # head dim
NCORES = 8
TC = 512          # t-chunk (matmul moving free dim)
ST = 128          # s-tile (partition dim)
NCHUNK = S // TC  # 4
NST = S // ST     # 16
NDT = D // 128    # 32 contraction tiles

QUERY_SCALE = 0.08838834764831845
SOFT_CAP = 50.0
SLIDING_WINDOW = 1024
ROPE_BASE = 10000.0

F32R = mybir.dt.float32r
F32 = mybir.dt.float32
BF16 = mybir.dt.bfloat16

TANH_SCALE = float(QUERY_SCALE / SOFT_CAP)
MASK_ADD = -2000.0  # tanh in [-1,1]; exp(50*(tanh-2000)) == 0 exactly

BF = ml_dtypes.bfloat16


def _build_program(active, nmask):
    """Build the SPMD Bass program.

    active: list over t-chunk ci of list of (j, mask_idx_or_None, c0, c1, q0)
            for mask-active 128-row s-tiles; [c0, c1) = active t window,
            q0 = start of the 128-wide partial-mask region (None if full).
    nmask:  number of distinct 128-wide additive mask tiles.
    """
    nc = bacc.Bacc("TRN2", target_bir_lowering=False, debug=False)

    xT = nc.dram_tensor("xT", [D, S], BF16, kind="ExternalInput").ap()
    w_all = nc.dram_tensor("w_all", [6, 128, NDT * 128], BF16,
                           kind="ExternalInput").ap()
    wo = nc.dram_tensor("wo", [G, H, D], BF16, kind="ExternalInput").ap()
    cs = nc.dram_tensor("cs", [128, 2, NCHUNK, TC], BF16, kind="ExternalInput").ap()
    consts = nc.dram_tensor("consts", [128, 384], F32R, kind="ExternalInput").ap()
    constsb = nc.dram_tensor("constsb", [128, 256], BF16, kind="ExternalInput").ap()
    masks = nc.dram_tensor("masks", [128, max(nmask, 1), 128], BF16,
                           kind="ExternalInput").ap()
    outp = nc.dram_tensor("outp", [S, D], BF16, kind="ExternalOutput").ap()

    Tanh = mybir.ActivationFunctionType.Tanh
    Exp = mybir.ActivationFunctionType.Exp

    from contextlib import ExitStack
    with tile.TileContext(nc) as tc:
        with ExitStack() as stack:
            def pool(name, bufs=1, space="SBUF"):
                return stack.enter_context(
                    tc.tile_pool(name=name, bufs=bufs, space=space))
            constp = pool("constp")
            ropedp = pool("ropedp")
            vsbp = pool("vsbp")
            wp = pool("wp")
            csp = pool("csp")
            mp = pool("mp")
            encp = pool("encp")
            xtp = pool("xtp", bufs=4)
            evp = pool("evp", bufs=4)
            vTp = pool("vTp", bufs=2)
            rtp = pool("rtp", bufs=3)
            tpp = pool("tpp", bufs=3)
            pp = pool("pp", bufs=26)
            sump = pool("sump")
            rcp = pool("rcp", bufs=2)
            wop = pool("wop", bufs=3)
            osbp = pool("osbp", bufs=2)
            psproj = pool("psproj", space="PSUM")
            pslp = pool("psl", space="PSUM")
            psep = pool("pse", space="PSUM")
            psop = pool("pso", space="PSUM")

            ct = constp.tile([128, 384], F32R)
            ctb = constp.tile([128, 256], BF16, name="ctb", tag="ctb")
            ident = ct[:, 256:384]
            swapmat = ctb[:, 0:128]
            allones = ctb[:, 128:256]

            qkr = [ropedp.tile([128, S], BF16, name=f"qkr{w}", tag=f"qkr{w}")
                   for w in range(5)]  # q0..q3, k
            v_sb = vsbp.tile([128, NST, 128], BF16)  # [s_lo, s_tile, h]
            encn = [encp.tile([128, S], BF16, name=f"encn{h}", tag=f"encn{h}")
                    for h in range(G)]

            # ---- resident small tensors -------------------------------
            cst = csp.tile([128, 2, NCHUNK, TC], BF16)
            mt = mp.tile([128, max(nmask, 1), 128], BF16)
            wts = [wp.tile([128, NDT, 128], BF16, name=f"wt{w}", tag=f"wt{w}")
                   for w in range(6)]
            w_src = [w_all[w].rearrange("p (dt h) -> p dt h", h=128)
                     for w in range(6)]

            # weight staging: dt tile 0 of pass-A weights first, then the rest
            passA = (4, 5, 0)   # k, v, q0
            passB = (1, 2, 3)
            for w in passA:
                nc.gpsimd.dma_start(out=wts[w][:, 0:1, :], in_=w_src[w][:, 0:1, :])
            boundsA = [1, 2, 4, 8, 16, 24, 32]
            for part in range(len(boundsA) - 1):
                dsl_ = slice(boundsA[part], boundsA[part + 1])
                for w in passA:
                    nc.gpsimd.dma_start(out=wts[w][:, dsl_, :],
                                        in_=w_src[w][:, dsl_, :])
            nc.gpsimd.dma_start(out=ctb, in_=constsb)
            nc.gpsimd.dma_start(out=cst, in_=cs)
            nc.gpsimd.dma_start(out=ct, in_=consts)
            boundsB = [0, 4, 8, 16, 24, 32]
            for part in range(len(boundsB) - 1):
                dsl_ = slice(boundsB[part], boundsB[part + 1])
                for w in passB:
                    nc.gpsimd.dma_start(out=wts[w][:, dsl_, :],
                                        in_=w_src[w][:, dsl_, :])
            nc.gpsimd.dma_start(out=mt, in_=masks)

            # ---------------- emission helpers ------------------------
            def emit_proj_pass(ci, ws):
                tsl = slice(ci * TC, (ci + 1) * TC)
                pss = {}
                for slot, w in enumerate(ws):
                    pss[w] = psproj.tile([128, TC], F32, name=f"pj{w}",
                                         tag=f"pj{slot}")
                for dt4 in range(NDT // 4):
                    xt = xtp.tile([128, 4, TC], BF16, name="xt", tag="xt")
                    src4 = xT[dt4 * 512:(dt4 + 1) * 512, tsl].rearrange(
                        "(k p) s -> p k s", k=4)
                    nc.sync.dma_start(out=xt, in_=src4)
                    for k in range(4):
                        dt_i = dt4 * 4 + k
                        for w in ws:
                            nc.tensor.matmul(pss[w], wts[w][:, dt_i, :],
                                             xt[:, k, :],
                                             start=(dt_i == 0),
                                             stop=(dt_i == NDT - 1))
                return pss

            def emit_rope(ci, pss, ws):
                """Evict + rope (q/k) or transpose (v) for the given weights."""
                tsl = slice(ci * TC, (ci + 1) * TC)
                cos_t = cst[:, 0, ci, :]
                sin_t = cst[:, 1, ci, :]
                for w in ws:
                    if w == 5:
                        vT = vTp.tile([128, TC], F32R, name="vT", tag="vT")
                        nc.scalar.copy(vT, pss[w])
                        for sti in range(4):
                            st = 4 * ci + sti
                            tp = pslp.tile([128, 128], F32R, name="tp",
                                           tag=f"psl{sti % 2}")
                            nc.tensor.transpose(tp, vT[:, sti * 128:(sti + 1) * 128],
                                                ident)
                            nc.vector.tensor_copy(v_sb[:, st, :], tp)
                    else:
                        ev = evp.tile([128, TC], BF16, name="ev", tag="ev")
                        nc.scalar.copy(ev, pss[w])
                        swp = pslp.tile([128, TC], F32, name="swp",
                                        tag=f"psl{w % 2}")
                        nc.tensor.matmul(swp, swapmat, ev, start=True, stop=True)
                        m1 = rtp.tile([128, TC], BF16, name="m1", tag="m1")
                        nc.vector.tensor_mul(m1, ev, cos_t)
                        m2 = rtp.tile([128, TC], BF16, name="m2", tag="m2")
                        nc.vector.tensor_mul(m2, swp, sin_t)
                        nc.vector.tensor_add(qkr[w][:, tsl], m1, m2)

            def emit_head_logits(ci, h):
                """Logits + softmax-numerator chain for one head; returns
                (ptiles dict, denom-tree root tile)."""
                ptiles = {}
                for (j, mi, c0, c1, q0) in active[ci]:
                    csl = slice(c0, c1)
                    ps = pslp.tile([128, TC], F32, name="psl_t", tag=f"psl{j % 2}")
                    nc.tensor.matmul(
                        ps[:, csl], qkr[4][:, j * 128:(j + 1) * 128],
                        qkr[h][:, ci * TC + c0:ci * TC + c1],
                        start=True, stop=True)
                    t1 = tpp.tile([128, TC], F32, name="t1", tag="t1")
                    nc.scalar.activation(t1[:, csl], ps[:, csl], Tanh,
                                         scale=TANH_SCALE)
                    if mi is not None:
                        nc.vector.tensor_add(t1[:, q0:q0 + 128],
                                             t1[:, q0:q0 + 128], mt[:, mi, :])
                    pt = pp.tile([128, TC], BF16, name="pt", tag="pt")
                    if c0 > 0:
                        nc.gpsimd.memset(pt[:, 0:c0], 0.0)
                    if c1 < TC:
                        nc.gpsimd.memset(pt[:, c1:TC], 0.0)
                    nc.scalar.activation(pt[:, csl], t1[:, csl], Exp,
                                         scale=SOFT_CAP)
                    ptiles[j] = pt
                # denominator: serial in-place accumulation on DVE, paced
                # by ACT exp production (each add consumes p_j as it lands)
                leaves = [ptiles[j] for j, _, _, _, _ in active[ci]]
                run = sump.tile([128, TC], BF16, name=f"run{h}",
                                tag=f"run{h % 2}")
                nc.vector.tensor_add(run, leaves[0], leaves[1])
                for p in leaves[2:]:
                    nc.vector.tensor_add(run, run, p)
                return ptiles, run

            def emit_head_finalize(ci, h, ptiles, run):
                tsl = slice(ci * TC, (ci + 1) * TC)
                js = [j for j, _, _, _, _ in active[ci]]
                dps = pslp.tile([128, TC], F32, name="dps", tag=f"psl{h % 2}")
                nc.tensor.matmul(dps, allones, run, start=True, stop=True)
                rec = rcp.tile([128, TC], F32, name="rec", tag="rec")
                nc.vector.reciprocal_approx_fast(out=rec, in_=dps)
                eps = psep.tile([128, TC], F32, name="eps", tag="eps")
                for idx, j in enumerate(js):
                    nc.tensor.matmul(eps, v_sb[:, j, :], ptiles[j],
                                     start=(idx == 0), stop=(idx == len(js) - 1))
                nc.vector.tensor_mul(encn[h][:, tsl], eps, rec)

            def prefetch_wo(dd):
                dsl = slice(dd * TC, (dd + 1) * TC)
                w4 = wop.tile([128, G, TC], BF16, name="w_t", tag="wo")
                nc.sync.dma_start(
                    out=w4, in_=wo[:, :, dsl].rearrange("g p d -> p g d"))
                return w4

            def emit_outproj_half(cp, dd, hf, w4):
                """Out-projection of chunk cp, dim slice dd, tt half hf."""
                dsl = slice(dd * TC, (dd + 1) * TC)
                ot2 = osbp.tile([128, 2, TC], BF16, name="ot", tag="ot")
                for ti, tt in enumerate(range(4 * cp + 2 * hf, 4 * cp + 2 * hf + 2)):
                    ps = psop.tile([128, TC], F32, name="pso_t", tag=f"pso{tt % 2}")
                    for h in range(G):
                        nc.tensor.matmul(
                            ps, encn[h][:, tt * 128:(tt + 1) * 128],
                            w4[:, h, :], start=(h == 0), stop=(h == G - 1))
                    nc.vector.tensor_copy(ot2[:, ti, :], ps)
                dst = outp[(4 * cp + 2 * hf) * 128:(4 * cp + 2 * hf + 2) * 128,
                           dsl].rearrange("(tt p) d -> p tt d", tt=2)
                nc.sync.dma_start(out=dst, in_=ot2)

            def emit_proj_units(ci):
                """Return thunks emitting proj+rope for chunk ci in pieces."""
                state = {}

                def pass_unit(ws, d4lo, d4hi, key):
                    def f():
                        tsl = slice(ci * TC, (ci + 1) * TC)
                        if key not in state:
                            state[key] = {
                                w: psproj.tile([128, TC], F32, name=f"pj{w}",
                                               tag=f"pj{slot}")
                                for slot, w in enumerate(ws)}
                        pss = state[key]
                        for dt4 in range(d4lo, d4hi):
                            xt = xtp.tile([128, 4, TC], BF16, name="xt", tag="xt")
                            src4 = xT[dt4 * 512:(dt4 + 1) * 512, tsl].rearrange(
                                "(k p) s -> p k s", k=4)
                            nc.sync.dma_start(out=xt, in_=src4)
                            for k in range(4):
                                dt_i = dt4 * 4 + k
                                for w in ws:
                                    nc.tensor.matmul(pss[w], wts[w][:, dt_i, :],
                                                     xt[:, k, :],
                                                     start=(dt_i == 0),
                                                     stop=(dt_i == NDT - 1))
                    return f
                units = []
                for lo in range(0, 8, 2):
                    units.append(pass_unit(passA, lo, lo + 2, "A"))
                units.append(lambda: emit_rope(ci, state.pop("A"), passA))
                for lo in range(0, 8, 2):
                    units.append(pass_unit(passB, lo, lo + 2, "B"))
                units.append(lambda: emit_rope(ci, state.pop("B"), passB))
                return units

            def emit_proj(ci):
                for u in emit_proj_units(ci):
                    u()

            def emit_attn(ci, cp, extra=None):
                """Attention for chunk ci. Fillers between heads: the
                out-projection halves of chunk cp (if not None) plus any
                extra thunks (e.g. proj units of a later chunk)."""
                units = []
                w4s = {}
                if cp is not None:
                    w4s[0] = prefetch_wo(0)
                    w4s[1] = prefetch_wo(1)
                    for dd in range(8):
                        for hf in range(2):
                            units.append((dd, hf))
                fillers = list(extra or [])
                fi = 0
                ui = 0

                def emit_units(n):
                    nonlocal ui, fi
                    for _ in range(n):
                        if ui < len(units):
                            dd, hf = units[ui]
                            if hf == 0:
                                if dd not in w4s:
                                    w4s[dd] = prefetch_wo(dd)
                                if dd + 1 < 8 and dd + 1 not in w4s:
                                    w4s[dd + 1] = prefetch_wo(dd + 1)
                            emit_outproj_half(cp, dd, hf, w4s[dd])
                            if hf == 1:
                                w4s.pop(dd)
                            ui += 1
                        elif fi < len(fillers):
                            fillers[fi]()
                            fi += 1

                prev = None
                for h in range(G):
                    ptiles, run = emit_head_logits(ci, h)
                    emit_units(2)
                    if prev is not None:
                        emit_head_finalize(ci, h - 1, *prev)
                    prev = (ptiles, run)
                    emit_units(2)
                emit_head_finalize(ci, G - 1, *prev)
                emit_units(len(units) + len(fillers))
                while fi < len(fillers):
                    fillers[fi]()
                    fi += 1

            # ---------------- program schedule -------------------------
            emit_proj(0)
            emit_proj(1)
            emit_proj(2)
            emit_attn(0, None, extra=emit_proj_units(3))
            emit_attn(1, 0)
            emit_attn(2, 1)
            emit_attn(3, 2)
            w4s = {0: prefetch_wo(0), 1: prefetch_wo(1)}
            for dd in range(8):
                if dd + 2 < 8:
                    w4s[dd + 2] = prefetch_wo(dd + 2)
                for hf in range(2):
                    emit_outproj_half(3, dd, hf, w4s[dd])
                w4s.pop(dd)

    nc.compile()
    return nc


def _host_prep(x, segment_pos, attn_mask):
    """Host-side preprocessing shared by all cores."""
    xT = np.ascontiguousarray(x[0].T).astype(BF)

    # rope tables, emulating the reference's float32 computation
    pos = segment_pos[0].astype(np.float32)                      # [S]
    fraction = (2.0 * np.arange(H // 2, dtype=np.float32)
                / np.float32(H)).astype(np.float32)
    timescale = (np.float32(ROPE_BASE) ** fraction).astype(np.float32)
    sinusoid = (pos[None, :] / timescale[:, None]).astype(np.float32)  # [64, S]
    cosT = np.cos(sinusoid).astype(np.float32)
    sinT = np.sin(sinusoid).astype(np.float32)
    cos2 = np.concatenate([cosT, cosT], axis=0)                  # [128, S]
    sin2 = np.concatenate([-sinT, sinT], axis=0)                 # [128, S]
    cs = np.ascontiguousarray(
        np.stack([cos2.reshape(128, NCHUNK, TC),
                  sin2.reshape(128, NCHUNK, TC)], axis=1)).astype(BF)

    # combined mask [T, S]
    cache_positions = np.arange(S, dtype=np.int64)[None, :]
    sp = segment_pos[0].astype(np.int64)[:, None]
    sliding = (cache_positions > sp - SLIDING_WINDOW) & \
              (cache_positions < sp + SLIDING_WINDOW)
    combined = np.asarray(attn_mask[0], dtype=bool) & sliding    # [T, S]

    # block classification at (128 s) x (512 t) granularity
    active = []
    mask_list = []
    mask_index = {}
    for ci in range(NCHUNK):
        row = []
        for j in range(NST):
            sub = combined[ci * TC:(ci + 1) * TC, j * ST:(j + 1) * ST]  # [t, s]
            if not sub.any():
                continue
            colact = sub.any(axis=1)              # [t]
            c0 = int(np.argmax(colact)) & ~7
            c1 = min(TC, (int(TC - np.argmax(colact[::-1])) + 7) & ~7)
            if sub[c0:c1].all():
                row.append((j, None, c0, c1, None))
                continue
            # partial columns: not all-active within [c0, c1)
            partial = ~sub[c0:c1].all(axis=1)     # [c1-c0]
            pidx = np.nonzero(partial)[0]
            q0 = (c0 + int(pidx[0])) & ~7
            q1 = c0 + int(pidx[-1]) + 1
            assert q1 - q0 <= 128, f"mask region too wide: {q0}..{q1}"
            q0 = min(q0, TC - 128)
            madd = np.where(sub.T[:, q0:q0 + 128], np.float32(0.0),
                            np.float32(MASK_ADD)).astype(BF)     # [s, 128]
            key = madd.tobytes()
            if key not in mask_index:
                mask_index[key] = len(mask_list)
                mask_list.append(madd)
            row.append((j, mask_index[key], c0, c1, q0))
        assert row, f"t-chunk {ci} attends to nothing"
        assert len(row) >= 2
        active.append(row)
    nmask = len(mask_list)
    if nmask:
        masks_host = np.ascontiguousarray(np.stack(mask_list, axis=1))  # [128,n,128]
    else:
        masks_host = np.zeros((128, 1, 128), dtype=BF)

    # consts: allones | swapmat | identity (f32r)
    allones = np.ones((128, 128), dtype=np.float32)
    swapmat = np.zeros((128, 128), dtype=np.float32)
    idx = np.arange(128)
    swapmat[idx, (idx + 64) % 128] = 1.0
    identity = np.eye(128, dtype=np.float32)
    consts = np.ascontiguousarray(
        np.concatenate([allones, swapmat, identity], axis=1))    # [128, 384]

    swapb = np.ascontiguousarray(
        np.concatenate([swapmat, allones], axis=1)).astype(BF)  # [128, 256]
    return xT, cs, active, nmask, masks_host, consts, swapb


def _core_weights(q_w, kv_w, out_w, c):
    qsel = np.asarray(q_w[G * c:G * (c + 1)], dtype=np.float32)   # [4,D,H]
    ksel = np.asarray(kv_w[0, c], dtype=np.float32)               # [D,H]
    vsel = np.asarray(kv_w[1, c], dtype=np.float32)               # [D,H]
    w6 = np.stack([qsel[0], qsel[1], qsel[2], qsel[3], ksel, vsel], axis=0)
    # [6, D, H] -> [6, 128(p), NDT*128] with (dt, h) contiguous per partition
    w_all_host = np.ascontiguousarray(
        w6.reshape(6, NDT, 128, 128).transpose(0, 2, 1, 3)
        .reshape(6, 128, NDT * 128)).astype(BF)
    wo_host = np.ascontiguousarray(
        np.asarray(out_w[G * c:G * (c + 1)], dtype=np.float32)).astype(BF)
    return w_all_host, wo_host


def kernel(x, segment_pos, attn_mask, q_w, kv_w, out_w, _trace=False, _repeat=1):
    x = np.asarray(x)
    segment_pos = np.asarray(segment_pos)
    attn_mask = np.asarray(attn_mask)
    q_w = np.asarray(q_w)
    kv_w = np.asarray(kv_w)
    out_w = np.asarray(out_w)
    assert x.shape == (1, S, D) and q_w.shape == (NQ, D, H), \
        f"kernel hardcoded for {(1, S, D)}, got {x.shape}"

    xT, cs, active, nmask, masks_host, consts, swapb = _host_prep(
        x, segment_pos, attn_mask)

    nc = _build_program(active, nmask)

    in_maps = []
    for c in range(NCORES):
        w_all_host, wo_host = _core_weights(q_w, kv_w, out_w, c)
        in_maps.append({
            "xT": xT, "w_all": w_all_host, "wo": wo_host, "cs": cs,
            "consts": consts, "constsb": swapb, "masks": masks_host,
        })

    res = run_bass_kernel_spmd(nc, in_maps, list(range(NCORES)), trace=_trace)
    kernel._last_exec_ns = res.exec_time_ns
    kernel._all_exec_ns = [res.exec_time_ns]
    for _ in range(_repeat - 1):
        r2 = run_bass_kernel_spmd(nc, in_maps, list(range(NCORES)), trace=_trace)
        kernel._all_exec_ns.append(r2.exec_time_ns)
        res = r2
    if _repeat > 1 and any(t for t in kernel._all_exec_ns if t):
        kernel._last_exec_ns = min(t for t in kernel._all_exec_ns if t)

    out = res.results[0]["outp"].astype(np.float32)
    for c in range(1, NCORES):
        out += res.results[c]["outp"].astype(np.float32)
    return out[None]  # [1, S, D]


kernel._last_exec_ns = None
